# revision 34
# baseline (speedup 1.0000x reference)
"""SSD detection post-processing (decode + softmax + per-class top-200 + NMS,
TTA-flip merge) as a Bass/Tile kernel for 8 Trainium2 NeuronCores.

Sharding: pure data parallel over the batch dim — core k handles images
8k..8k+7 (= 168 (image,class) pairs per core).

Per-core pipeline (all on device):
  1. decode both views' boxes (flip view 2), store to a DRAM box table
  2. softmax probs in prior-major layout (no max-subtraction: logits are
     bounded); 32x32 stream-transpose + SBUF-SBUF DMA reassembly into
     pair-major score rows [pair, 17664]
  3. chunk-max (L=8) -> M [pair, 2208]; chunk ids are packed into the low
     12 mantissa bits of M (bit-complemented so exact ties keep ascending
     chunk order), so 25 rounds of max8/match_replace extract both the 200
     largest chunk maxes AND their ids with no find_index8 pass. The
     packing quantization (2^-11 relative) only perturbs which chunks make
     the top-200 cut at the boundary — scores themselves stay exact.
  4. indirect-DMA gather of those chunks -> pool [pair, 1600]; 25 exact
     max8/max_index/match_replace rounds give the sorted top-200 + their
     pool slots
  5. pool slot -> chunk id via a u16 one-hot select; box rows arrive by
     per-rank indirect gathers
  6. upper-triangle IoU mask (iou > 0.45 as 1.45*inter - 0.45*(ai+aj) > 0),
     emitted two sort-rounds behind the gathers so the DVE never waits
  7. greedy NMS via a running suppression vector: keep[i] = (supv[i]==0);
     supv[k>i] max= keep[i]*Sm[i, k>i]
  8. sorted rows + keep flags DMA out; the host compacts survivors
     (replaces 400 per-rank scatter DMAs that dominated the old tail)

The 168 pairs exceed the 128 SBUF partitions, so phase 2 runs as two
partition tiles (126 + 42 pairs) whose instruction streams interleave
op-by-op to hide dependency latency.
"""

import numpy as np

B = 64
N = 8732
C = 21
NPAD = 8832            # priors padded to 69*128
SLOTS = NPAD // 128    # 69
W2 = 2 * NPAD          # 17664 score columns per pair
IMG = 8                # images per core
PAIRS = IMG * C        # 168 pairs per core
CHUNK = 8
NCHUNK = W2 // CHUNK   # 2208
ROUNDS = 25            # 25*8 = 200
K = 200
SKEW = 2               # IoU runs this many sort-rounds behind the gathers
TILES = ((0, 126), (126, 42))   # (pair offset, pair count) per partition tile


def build_nc(level=99):
    import concourse.bacc as bacc
    import concourse.bass as bass
    import concourse.mybir as mybir
    from concourse.bass import IndirectOffsetOnAxis
    from concourse.tile import TileContext

    f32 = mybir.dt.float32
    u32 = mybir.dt.uint32
    u16 = mybir.dt.uint16
    u8 = mybir.dt.uint8
    Alu = mybir.AluOpType
    Act = mybir.ActivationFunctionType
    Ax = mybir.AxisListType

    nc = bacc.Bacc()

    def TT(out, in0, in1, op):
        # TensorTensor's ISA struct can't encode multiple sync waits; emit
        # every tensor-tensor op as (in0 bypass 0.0) op in1 instead.
        nc.vector.scalar_tensor_tensor(
            out=out, in0=in0, scalar=0.0, in1=in1, op0=Alu.bypass, op1=op,
        )

    loc1 = nc.declare_dram_parameter("loc1", [IMG, NPAD, 4], f32, isOutput=False)
    loc2 = nc.declare_dram_parameter("loc2", [IMG, NPAD, 4], f32, isOutput=False)
    conf1 = nc.declare_dram_parameter("conf1", [IMG, NPAD, C], f32, isOutput=False)
    conf2 = nc.declare_dram_parameter("conf2", [IMG, NPAD, C], f32, isOutput=False)
    dbox = nc.declare_dram_parameter("dbox", [NPAD, 4], f32, isOutput=False)
    # aux[pair] = (img*W2 base row into box table, unused)
    aux = nc.declare_dram_parameter("aux", [PAIRS, 2], u32, isOutput=False)
    outs = nc.declare_dram_parameter("outs", [PAIRS, K], f32, isOutput=True)
    outb = nc.declare_dram_parameter("outb", [PAIRS, K, 4], f32, isOutput=True)
    outk = nc.declare_dram_parameter("outk", [PAIRS, K], u8, isOutput=True)

    scoresD = nc.dram_tensor("scoresD", [PAIRS * NCHUNK, CHUNK], f32)
    # prior-ordered box table: row = (img*2+v)*NPAD + prior
    boxesD = nc.dram_tensor("boxesD", [IMG * W2, 4], f32)

    with TileContext(nc) as tc:
        with tc.tile_pool(name="Mpool", bufs=1) as mp:
            M_tiles = [
                mp.tile([cnt, NCHUNK], f32, tag=f"M{ti}", name=f"M{ti}")
                for ti, (off, cnt) in enumerate(TILES)
            ]
            # ---------------- phase 1: scores + boxes -----------------------
            with (
                tc.tile_pool(name="persist", bufs=1) as pp,
                tc.tile_pool(name="work", bufs=1) as wp,
                tc.tile_pool(name="workdb", bufs=2) as wdb,
            ):
                SA = pp.tile([126, W2], f32, tag="SA")
                SB = pp.tile([42, W2], f32, tag="SB")
                S_tiles = (SA, SB)

                pr_t = wp.tile([128, SLOTS, 32], f32, tag="pr")
                # pad columns feed transpose lanes that are never read, but
                # must not be uninitialized SBUF
                nc.gpsimd.memset(pr_t[:, :, C:], 0.0)

                dbox_t = pp.tile([128, SLOTS, 4], f32, tag="dbox")
                nc.sync.dma_start(
                    out=dbox_t[:, :, :],
                    in_=dbox.rearrange("(p s) c -> p s c", s=SLOTS),
                )

                for img in range(IMG):
                    for v, (locp, confp) in enumerate(
                        ((loc1, conf1), (loc2, conf2))
                    ):
                        # ---- decode ----
                        loc_t = wdb.tile([128, SLOTS, 4], f32, tag="loc")
                        nc.sync.dma_start(
                            out=loc_t[:, :, :],
                            in_=locp[img].rearrange("(p s) c -> p s c", s=SLOTS),
                        )
                        box_t = wdb.tile([128, SLOTS, 4], f32, tag="box")
                        wh_t = wp.tile([128, SLOTS, 2], f32, tag="wh")
                        t1_t = wp.tile([128, SLOTS, 2], f32, tag="dtmp")
                        # wh = dbox_wh * exp(0.2*loc_wh)
                        nc.scalar.activation(
                            wh_t[:, :, :], loc_t[:, :, 2:4], Act.Exp, scale=0.2
                        )
                        TT(
                            out=wh_t[:, :, :], in0=wh_t[:, :, :],
                            in1=dbox_t[:, :, 2:4], op=Alu.mult,
                        )
                        # cxy = dbox_xy + (loc_xy*0.1)*dbox_xy
                        nc.vector.tensor_scalar_mul(
                            t1_t[:, :, :], loc_t[:, :, :2], 0.1
                        )
                        TT(
                            out=t1_t[:, :, :], in0=t1_t[:, :, :],
                            in1=dbox_t[:, :, :2], op=Alu.mult,
                        )
                        TT(
                            out=t1_t[:, :, :], in0=t1_t[:, :, :],
                            in1=dbox_t[:, :, :2], op=Alu.add,
                        )
                        # mn = cxy - 0.5*wh ; mx = mn + wh
                        nc.vector.tensor_scalar_mul(
                            box_t[:, :, 2:4], wh_t[:, :, :], 0.5
                        )
                        TT(
                            out=box_t[:, :, 0:2], in0=t1_t[:, :, :],
                            in1=box_t[:, :, 2:4], op=Alu.subtract,
                        )
                        TT(
                            out=box_t[:, :, 2:4], in0=box_t[:, :, 0:2],
                            in1=wh_t[:, :, :], op=Alu.add,
                        )
                        if v == 1:
                            # flip: x1' = 1-x2, x2' = 1-x1
                            fx_t = wp.tile([128, SLOTS, 2], f32, tag="fx")
                            nc.vector.tensor_scalar(
                                fx_t[:, :, 0:1], box_t[:, :, 2:3], -1.0, 1.0,
                                op0=Alu.mult, op1=Alu.add,
                            )
                            nc.vector.tensor_scalar(
                                fx_t[:, :, 1:2], box_t[:, :, 0:1], -1.0, 1.0,
                                op0=Alu.mult, op1=Alu.add,
                            )
                            nc.vector.tensor_copy(
                                box_t[:, :, 0:1], fx_t[:, :, 0:1]
                            )
                            nc.vector.tensor_copy(
                                box_t[:, :, 2:3], fx_t[:, :, 1:2]
                            )
                        # boxesD rows (img*2+v)*NPAD + prior: contiguous per
                        # partition (prior = p*SLOTS + s), one clean DMA
                        nc.scalar.dma_start(
                            out=boxesD[(img * 2 + v) * NPAD:
                                       (img * 2 + v + 1) * NPAD, :].rearrange(
                                "(p s) c -> p s c", s=SLOTS
                            ),
                            in_=box_t[:, :, :],
                        )

                        # ---- softmax (prior-major) ----
                        cf_t = wdb.tile([128, SLOTS, C], f32, tag="cf")
                        nc.scalar.dma_start(
                            out=cf_t[:, :, :],
                            in_=confp[img].rearrange("(p s) c -> p s c", s=SLOTS),
                        )
                        mx_t = wp.tile([128, SLOTS], f32, tag="mx")
                        nc.vector.tensor_reduce(
                            out=mx_t[:, :], in_=cf_t[:, :, :], axis=Ax.X,
                            op=Alu.max,
                        )
                        TT(
                            out=cf_t[:, :, :], in0=cf_t[:, :, :],
                            in1=mx_t[:, :, None].to_broadcast([128, SLOTS, C]),
                            op=Alu.subtract,
                        )
                        nc.scalar.activation(
                            pr_t[:, :, :C], cf_t[:, :, :], Act.Exp
                        )
                        sm_t = wp.tile([128, SLOTS], f32, tag="sm")
                        nc.vector.tensor_reduce(
                            out=sm_t[:, :], in_=pr_t[:, :, :C], axis=Ax.X,
                            op=Alu.add,
                        )
                        nc.vector.reciprocal(sm_t[:, :], sm_t[:, :])
                        TT(
                            out=pr_t[:, :, :C], in0=pr_t[:, :, :C],
                            in1=sm_t[:, :, None].to_broadcast([128, SLOTS, C]),
                            op=Alu.mult,
                        )
                        # ---- 32x32 block transpose ----
                        tr_t = wdb.tile([128, SLOTS, 32], f32, tag="tr")
                        nc.vector.transpose(
                            out=tr_t[:, :, :].rearrange("p s c -> p (s c)"),
                            in_=pr_t[:, :, :].rearrange("p s c -> p (s c)"),
                        )
                        # ---- SBUF->SBUF DMA into pair-major rows ----
                        if img < 6:
                            dst, row0 = SA, img * C
                        else:
                            dst, row0 = SB, (img - 6) * C
                        for pg in range(4):
                            nc.sync.dma_start(
                                out=dst[row0:row0 + C,
                                        v * NPAD + pg * (SLOTS * 32):
                                        v * NPAD + (pg + 1) * (SLOTS * 32)],
                                in_=tr_t[pg * 32: pg * 32 + C, :, :].rearrange(
                                    "c s l -> c (s l)"
                                ),
                            )

                # big copy of pair-major scores to DRAM + chunk max
                for (off, cnt), st, M_t in zip(TILES, S_tiles, M_tiles):
                    nc.scalar.dma_start(
                        out=scoresD[off * NCHUNK:(off + cnt) * NCHUNK, :],
                        in_=st[:, :].rearrange("p (n k) -> p n k", k=CHUNK),
                    )
                    nc.vector.tensor_reduce(
                        out=M_t[:, :],
                        in_=st[:, :].rearrange("p (n k) -> p n k", k=CHUNK),
                        axis=Ax.X, op=Alu.max,
                    )
            # persist pool (SA/SB) freed here

            # -------- phase 2: selection + NMS, tiles interleaved -----------
            if level < 2:
                nc.compile()
                return nc
            with tc.tile_pool(name="sel", bufs=1) as sp:
                D = []   # per-tile working tiles
                for ti, (off, cnt) in enumerate(TILES):
                    d = {}
                    d["cids"] = sp.tile([cnt, K], u32, tag=f"ci{ti}", name=f"ci{ti}")
                    d["pb"] = sp.tile([cnt, K], u16, tag=f"pb{ti}", name=f"pb{ti}")
                    d["t0"] = sp.tile([cnt, K], u32, tag=f"t0{ti}", name=f"t0{ti}")
                    d["t1"] = sp.tile([cnt, K], u32, tag=f"t1{ti}", name=f"t1{ti}")
                    d["t2"] = sp.tile([cnt, K], u32, tag=f"t2{ti}", name=f"t2{ti}")
                    d["t3"] = sp.tile([cnt, K], u32, tag=f"t3{ti}", name=f"t3{ti}")
                    d["c32"] = sp.tile([cnt, 8], u32, tag=f"c3{ti}", name=f"c3{ti}")
                    d["v8"] = sp.tile([cnt, 8], f32, tag=f"v8{ti}", name=f"v8{ti}")
                    # f32 per-partition bases (values < 2^24, exact): AP
                    # scalars for arithmetic tensor_scalar ops must be f32
                    d["base"] = sp.tile([cnt, 1], f32, tag=f"ba{ti}", name=f"ba{ti}")
                    nc.gpsimd.iota(
                        d["base"][:, :], pattern=[[0, 1]],
                        base=off * NCHUNK, channel_multiplier=NCHUNK,
                        allow_small_or_imprecise_dtypes=True,
                    )
                    # descending 12-bit id pattern for the mantissa packing
                    d["revi"] = sp.tile([cnt, NCHUNK], u32, tag=f"rv{ti}", name=f"rv{ti}")
                    nc.gpsimd.iota(
                        d["revi"][:, :], pattern=[[-1, NCHUNK]],
                        base=4095, channel_multiplier=0,
                    )
                    d["iot"] = sp.tile([cnt, K], u16, tag=f"io{ti}", name=f"io{ti}")
                    nc.gpsimd.iota(
                        d["iot"][:, :], pattern=[[1, K]], base=0,
                        channel_multiplier=0,
                    )
                    d["gidx"] = sp.tile([cnt, K], u32, tag=f"gi{ti}", name=f"gi{ti}")
                    d["pool"] = sp.tile([cnt, K, CHUNK], f32, tag=f"po{ti}", name=f"po{ti}")
                    d["aux"] = sp.tile([cnt, 2], u32, tag=f"ax{ti}", name=f"ax{ti}")
                    nc.sync.dma_start(
                        out=d["aux"][:, :], in_=aux[off:off + cnt, :]
                    )
                    d["auxf"] = sp.tile([cnt, 1], f32, tag=f"af{ti}", name=f"af{ti}")
                    nc.vector.tensor_copy(d["auxf"][:, :], d["aux"][:, 0:1])
                    D.append(d)

                # ---- pack chunk ids into the low 12 mantissa bits of M ----
                for ti, (off, cnt) in enumerate(TILES):
                    M_t, d = M_tiles[ti], D[ti]
                    Mu = M_t[:, :].bitcast(u32)
                    nc.vector.tensor_scalar(
                        Mu, Mu, 12, 12,
                        op0=Alu.logical_shift_right, op1=Alu.logical_shift_left,
                    )
                    nc.vector.tensor_tensor(
                        out=Mu, in0=Mu, in1=d["revi"][:, :], op=Alu.bitwise_or
                    )

                # ---- top-200 chunk extraction, pool gather fused in ----
                # tile1 runs completely first: its 200 pool gathers clear the
                # Q7 early so its sort/IoU/scan overlap tile0's gather backlog
                for r in range(ROUNDS):
                    for ti in (1, 0):
                        off, cnt = TILES[ti]
                        M_t, d = M_tiles[ti], D[ti]
                        nc.vector.max(out=d["v8"][:, :], in_=M_t[:, :])
                        nc.vector.match_replace(
                            out=M_t[:, :], in_to_replace=d["v8"][:, :],
                            in_values=M_t[:, :], imm_value=-1.0,
                        )
                        # cid = (v8 & 0xFFF) ^ 0xFFF  (undo the complement)
                        nc.vector.tensor_scalar(
                            d["cids"][:, 8 * r:8 * r + 8],
                            d["v8"][:, :].bitcast(u32), 0xFFF, 0xFFF,
                            op0=Alu.bitwise_and, op1=Alu.bitwise_xor,
                        )
                        nc.vector.tensor_scalar(
                            d["gidx"][:, 8 * r:8 * r + 8],
                            d["cids"][:, 8 * r:8 * r + 8],
                            d["base"][:, :], None, op0=Alu.add,
                        )
                        if ti == 1:
                            # tile0's pool gathers are deferred into tile1's
                            # sort loop so tile1's box gathers aren't stuck
                            # behind them in the in-order Q7 stream
                            for j in range(8):
                                sg = 8 * r + j
                                nc.gpsimd.indirect_dma_start(
                                    out=d["pool"][:, sg, :], out_offset=None,
                                    in_=scoresD[:, :],
                                    in_offset=IndirectOffsetOnAxis(
                                        ap=d["gidx"][:, sg:sg + 1], axis=0),
                                )
                if level < 3:
                    nc.compile()
                    return nc

                # ---- chunk id -> prior-base decomposition (pb) ----
                # cid = v*1104 + pg*276 + s*4 + l38; box row =
                # imgv*NPAD + (pg*32 + l38*8 + lane)*69 + s, so
                # pb = v*NPAD + pg*2208 + l38*552 + s  (fits u16)
                for ti, (off, cnt) in enumerate(TILES):
                    d = D[ti]

                    def ts(out_, in_, s1, s2, op0, op1=None):
                        kw = {} if op1 is None else {"op1": op1}
                        nc.vector.tensor_scalar(out_, in_, s1, s2, op0=op0, **kw)

                    def stt(out_, in0, s, in1, op0, op1):
                        nc.vector.scalar_tensor_tensor(
                            out=out_, in0=in0, scalar=s, in1=in1,
                            op0=op0, op1=op1,
                        )

                    cids = d["cids"][:, :]
                    t0, t1, t2, t3 = (d[k][:, :] for k in ("t0", "t1", "t2", "t3"))
                    ts(t0, cids, 1104, None, Alu.is_ge)                  # v
                    stt(t1, t0, -1104.0, cids, Alu.mult, Alu.add)        # c2
                    ts(t2, t1, 276, None, Alu.is_ge)                     # p1
                    ts(t3, t1, 552, None, Alu.is_ge)                     # p2
                    TT(out=t2, in0=t2, in1=t3, op=Alu.add)
                    ts(t3, t1, 828, None, Alu.is_ge)                     # p3
                    TT(out=t2, in0=t2, in1=t3, op=Alu.add)               # pg
                    stt(t1, t2, -276.0, t1, Alu.mult, Alu.add)           # c3
                    ts(t3, t1, 2, None, Alu.logical_shift_right)         # s
                    ts(t1, t1, 3, None, Alu.bitwise_and)                 # l38
                    stt(t1, t1, 552.0, t3, Alu.mult, Alu.add)            # +s
                    stt(t1, t2, 2208.0, t1, Alu.mult, Alu.add)           # +pg
                    stt(t1, t0, float(NPAD), t1, Alu.mult, Alu.add)      # +v
                    nc.vector.tensor_copy(d["pb"][:, :], t1)

                # ---- fused: pool sort -> one-hot sigma -> box gather -> IoU
                for ti, (off, cnt) in enumerate(TILES):
                    d = D[ti]
                    d["sorted"] = sp.tile([cnt, K], f32, tag=f"so{ti}", name=f"so{ti}")
                    d["ps"] = sp.tile([cnt, K], u32, tag=f"ps{ti}", name=f"ps{ti}")
                    d["sh"] = sp.tile([cnt, K], u16, tag=f"sh{ti}", name=f"sh{ti}")
                    d["oh"] = sp.tile([cnt, 8, K], u16, tag=f"oh{ti}", name=f"oh{ti}")
                    d["ohm"] = sp.tile([cnt, 8, K], u16, tag=f"om{ti}", name=f"om{ti}")
                    d["csel"] = sp.tile([cnt, K], u16, tag=f"cs{ti}", name=f"cs{ti}")
                    d["lane"] = sp.tile([cnt, K], u32, tag=f"la{ti}", name=f"la{ti}")
                    d["sig"] = sp.tile([cnt, K], u32, tag=f"sg{ti}", name=f"sg{ti}")
                    d["bx"] = sp.tile([cnt, K, 4], f32, tag=f"bx{ti}", name=f"bx{ti}")
                    d["ar"] = sp.tile([cnt, K], f32, tag=f"ar{ti}", name=f"ar{ti}")
                    d["w0"] = sp.tile([cnt, K], f32, tag=f"w0{ti}", name=f"w0{ti}")
                    d["Sm"] = sp.tile([cnt, K, K], u8, tag=f"Sm{ti}", name=f"Sm{ti}")
                    d["xa"] = sp.tile([cnt, K, 8], f32, tag=f"xa{ti}", name=f"xa{ti}")
                    d["xb"] = sp.tile([cnt, K, 8], f32, tag=f"xb{ti}", name=f"xb{ti}")
                    d["xc"] = sp.tile([cnt, K, 8], f32, tag=f"xc{ti}", name=f"xc{ti}")

                def emit_ar_iou(ti, rq):
                    # areas + IoU column-block for sort round rq
                    q0, q1 = 8 * rq, 8 * rq + 8
                    if True:
                        off, cnt = TILES[ti]
                        d = D[ti]
                        bxg = d["bx"][:, q0:q1, :]
                        TT(
                            out=d["w0"][:, q0:q1], in0=bxg[:, :, 2],
                            in1=bxg[:, :, 0], op=Alu.subtract,
                        )
                        TT(
                            out=d["ar"][:, q0:q1], in0=bxg[:, :, 3],
                            in1=bxg[:, :, 1], op=Alu.subtract,
                        )
                        TT(
                            out=d["ar"][:, q0:q1], in0=d["ar"][:, q0:q1],
                            in1=d["w0"][:, q0:q1], op=Alu.mult,
                        )
                        # IoU column-block: S[a, b] for a < r1, b in [r0, r1)
                        A = q1
                        sh3 = [cnt, A, 8]
                        bx = d["bx"]
                        xa = d["xa"][:, :A, :]
                        xb = d["xb"][:, :A, :]
                        xc = d["xc"][:, :A, :]
                        TT(
                            out=xc,
                            in0=bx[:, :A, 1:2].to_broadcast(sh3),
                            in1=bx[:, None, q0:q1, 1].to_broadcast(sh3),
                            op=Alu.max,
                        )
                        TT(
                            out=xb,
                            in0=bx[:, :A, 3:4].to_broadcast(sh3),
                            in1=bx[:, None, q0:q1, 3].to_broadcast(sh3),
                            op=Alu.min,
                        )
                        TT(out=xb, in0=xb, in1=xc, op=Alu.subtract)
                        nc.scalar.activation(xb, xb, Act.Relu)
                        TT(
                            out=xa,
                            in0=bx[:, :A, 0:1].to_broadcast(sh3),
                            in1=bx[:, None, q0:q1, 0].to_broadcast(sh3),
                            op=Alu.max,
                        )
                        TT(
                            out=xc,
                            in0=bx[:, :A, 2:3].to_broadcast(sh3),
                            in1=bx[:, None, q0:q1, 2].to_broadcast(sh3),
                            op=Alu.min,
                        )
                        TT(out=xa, in0=xc, in1=xa, op=Alu.subtract)
                        nc.scalar.activation(xa, xa, Act.Relu)
                        TT(out=xa, in0=xa, in1=xb, op=Alu.mult)
                        TT(
                            out=xb,
                            in0=d["ar"][:, :A, None].to_broadcast(sh3),
                            in1=d["ar"][:, None, q0:q1].to_broadcast(sh3),
                            op=Alu.add,
                        )
                        nc.vector.scalar_tensor_tensor(
                            out=xa, in0=xb, scalar=-0.45 / 1.45, in1=xa,
                            op0=Alu.mult, op1=Alu.add,
                        )
                        nc.vector.tensor_scalar(
                            d["Sm"][:, :A, q0:q1], xa, 0.0, None,
                            op0=Alu.is_gt,
                        )

                # NMS scan tiles (allocated up front; tile1's scan
                # interleaves into tile0's sort rounds)
                for ti, (off, cnt) in enumerate(TILES):
                    d = D[ti]
                    d["keep"] = sp.tile([cnt, K], u8, tag=f"ke{ti}", name=f"ke{ti}")
                    d["supv"] = sp.tile([cnt, K], u8, tag=f"sv{ti}", name=f"sv{ti}")
                    nc.vector.memset(d["supv"][:, :], 0)

                def emit_scan_steps(ti, steps):
                    d = D[ti]
                    for i in steps:
                        nc.vector.scalar_tensor_tensor(
                            out=d["supv"][:, i + 1:],
                            in0=d["Sm"][:, i, i + 1:],
                            scalar=d["supv"][:, i:i + 1],
                            in1=d["supv"][:, i + 1:],
                            op0=Alu.is_gt, op1=Alu.max,
                        )

                def emit_sort_round(ti, r):
                    r0, r1 = 8 * r, 8 * r + 8
                    # sort round r only needs pool slots < 64r+64: rank
                    # k's chunk has chunk-max rank <= k, i.e. slot < 8k+8.
                    # +24 chunks of margin absorb rank perturbation from the
                    # mantissa packing (2^-11) and exact-tie groups.
                    npre = min(K, 8 * r + 8 + 24)
                    off, cnt = TILES[ti]
                    d = D[ti]
                    poolf = d["pool"][:, :npre, :].rearrange(
                        "p n k -> p (n k)")
                    nc.vector.max(
                        out=d["sorted"][:, r0:r1], in_=poolf
                    )
                    nc.vector.max_index(
                        out=d["ps"][:, r0:r1],
                        in_max=d["sorted"][:, r0:r1], in_values=poolf,
                    )
                    nc.vector.match_replace(
                        out=poolf,
                        in_to_replace=d["sorted"][:, r0:r1],
                        in_values=poolf, imm_value=-1.0,
                    )
                    # slot -> chunk id (u16 one-hot over the 200 slots);
                    # bitvec shift can't cast u32->u16, so shift then copy
                    nc.vector.tensor_scalar(
                        d["c32"][:, :], d["ps"][:, r0:r1], 3, None,
                        op0=Alu.logical_shift_right,
                    )
                    nc.vector.tensor_copy(
                        d["sh"][:, r0:r1], d["c32"][:, :]
                    )
                    sh3 = [cnt, 8, K]
                    TT(
                        out=d["oh"][:, :, :],
                        in0=d["sh"][:, r0:r1, None].to_broadcast(sh3),
                        in1=d["iot"][:, None, :].to_broadcast(sh3),
                        op=Alu.is_equal,
                    )
                    TT(
                        out=d["ohm"][:, :, :], in0=d["oh"][:, :, :],
                        in1=d["pb"][:, None, :].to_broadcast(sh3),
                        op=Alu.mult,
                    )
                    nc.vector.tensor_reduce(
                        out=d["csel"][:, r0:r1], in_=d["ohm"][:, :, :],
                        axis=Ax.X, op=Alu.max,
                    )
                    # box row = imgv*NPAD + pb + lane*69
                    nc.vector.tensor_scalar(
                        d["lane"][:, r0:r1], d["ps"][:, r0:r1], 7, None,
                        op0=Alu.bitwise_and,
                    )
                    nc.vector.scalar_tensor_tensor(
                        out=d["sig"][:, r0:r1], in0=d["lane"][:, r0:r1],
                        scalar=float(SLOTS), in1=d["csel"][:, r0:r1],
                        op0=Alu.mult, op1=Alu.add,
                    )
                    nc.vector.tensor_scalar(
                        d["sig"][:, r0:r1], d["sig"][:, r0:r1],
                        d["auxf"][:, :], None, op0=Alu.add,
                    )
                    # box gathers queue as soon as this tile's sig lands
                    for sg in range(r0, r1):
                        nc.gpsimd.indirect_dma_start(
                            out=d["bx"][:, sg, :], out_offset=None,
                            in_=boxesD[:, :],
                            in_offset=IndirectOffsetOnAxis(
                                ap=d["sig"][:, sg:sg + 1], axis=0),
                        )

                def finish_tile(ti):
                    off, cnt = TILES[ti]
                    d = D[ti]
                    nc.sync.dma_start(
                        out=outs[off:off + cnt, :], in_=d["sorted"][:, :]
                    )
                    nc.scalar.dma_start(
                        out=outb[off:off + cnt, :, :], in_=d["bx"][:, :, :]
                    )

                def finish_scan(ti):
                    off, cnt = TILES[ti]
                    d = D[ti]
                    nc.vector.tensor_scalar(
                        d["keep"][:, :], d["supv"][:, :], 0, None,
                        op0=Alu.is_equal,
                    )
                    nc.sync.dma_start(
                        out=outk[off:off + cnt, :], in_=d["keep"][:, :]
                    )

                # tile1 sort phase; tile0's pool gathers stream alongside
                d0 = D[0]
                for r in range(ROUNDS):
                    emit_sort_round(1, r)
                    for sg in range(8 * r, 8 * r + 8):
                        nc.gpsimd.indirect_dma_start(
                            out=d0["pool"][:, sg, :], out_offset=None,
                            in_=scoresD[:, :],
                            in_offset=IndirectOffsetOnAxis(
                                ap=d0["gidx"][:, sg:sg + 1], axis=0),
                        )
                    if r >= SKEW:
                        emit_ar_iou(1, r - SKEW)
                for rq in range(ROUNDS - SKEW, ROUNDS):
                    emit_ar_iou(1, rq)
                finish_tile(1)
                if level < 6:
                    nc.compile()
                    return nc

                # tile0 sort phase with tile1's scan steps interleaved
                t1_steps = list(range(K - 1))
                per_round = (len(t1_steps) + ROUNDS - 1) // ROUNDS
                for r in range(ROUNDS):
                    emit_sort_round(0, r)
                    lo = r * per_round
                    emit_scan_steps(1, t1_steps[lo:lo + per_round])
                    if r >= SKEW:
                        emit_ar_iou(0, r - SKEW)
                for rq in range(ROUNDS - SKEW, ROUNDS):
                    emit_ar_iou(0, rq)
                finish_tile(0)
                finish_scan(1)

                # tile0 scan
                emit_scan_steps(0, range(K - 1))
                finish_scan(0)
    nc.compile()
    return nc


def _prep_core_inputs(loc_b, conf_b, loc2_b, conf2_b, dbox):
    """Pad per-core inputs to NPAD priors; build aux table."""
    pad = NPAD - N
    locp = np.pad(loc_b, ((0, 0), (0, pad), (0, 0)))
    loc2p = np.pad(loc2_b, ((0, 0), (0, pad), (0, 0)))
    cpad = np.zeros((conf_b.shape[0], pad, C), np.float32)
    cpad[:, :, 0] = 40.0
    cpad[:, :, 1:] = -40.0
    confp = np.concatenate([conf_b, cpad], axis=1)
    conf2p = np.concatenate([conf2_b, cpad], axis=1)
    dpad = np.zeros((pad, 4), np.float32)
    dpad[:, 2:] = 1e-3
    dboxp = np.concatenate([dbox, dpad], axis=0)
    aux = np.zeros((PAIRS, 2), np.uint32)
    for p in range(PAIRS):
        aux[p, 0] = (p // C) * W2
    return {
        "loc1": np.ascontiguousarray(locp, np.float32),
        "loc2": np.ascontiguousarray(loc2p, np.float32),
        "conf1": np.ascontiguousarray(confp, np.float32),
        "conf2": np.ascontiguousarray(conf2p, np.float32),
        "dbox": np.ascontiguousarray(dboxp, np.float32),
        "aux": aux,
    }


def _compact_core(outs, outb, outk):
    """Host-side NMS survivor compaction for one core's outputs."""
    rows = np.concatenate([outs[:, :, None], outb], axis=2)  # [PAIRS, K, 5]
    keep = outk.astype(bool)
    keep[0::C, :] = False   # background class never processed
    pos = np.cumsum(keep, axis=1) - 1
    out = np.zeros((PAIRS, K, 5), np.float32)
    pi, ri = np.nonzero(keep)
    out[pi, pos[keep], :] = rows[pi, ri, :]
    return out


def kernel(loc_data, conf_data, loc_data2, conf_data2, dbox_list):
    from concourse.bass_utils import run_bass_kernel_spmd

    loc_data = np.asarray(loc_data, np.float32)
    conf_data = np.asarray(conf_data, np.float32)
    loc_data2 = np.asarray(loc_data2, np.float32)
    conf_data2 = np.asarray(conf_data2, np.float32)
    dbox_list = np.asarray(dbox_list, np.float32)

    nc = build_nc()
    in_maps = []
    for k in range(8):
        sl = slice(k * IMG, (k + 1) * IMG)
        in_maps.append(
            _prep_core_inputs(
                loc_data[sl], conf_data[sl], loc_data2[sl], conf_data2[sl],
                dbox_list,
            )
        )
    res = run_bass_kernel_spmd(nc, in_maps, list(range(8))).results
    outs = []
    for k in range(8):
        o = _compact_core(
            np.asarray(res[k]["outs"]), np.asarray(res[k]["outb"]),
            np.asarray(res[k]["outk"]),
        )
        outs.append(o.reshape(IMG, C, K, 5))
    return np.concatenate(outs, axis=0)


# revision 35
# speedup vs baseline: 1.0132x; 1.0132x over previous
"""SSD detection post-processing (decode + softmax + per-class top-200 + NMS,
TTA-flip merge) as a Bass/Tile kernel for 8 Trainium2 NeuronCores.

Sharding: pure data parallel over the batch dim — core k handles images
8k..8k+7 (= 168 (image,class) pairs per core).

Per-core pipeline (all on device):
  1. decode both views' boxes (flip view 2), store to a DRAM box table
  2. softmax probs in prior-major layout (no max-subtraction: logits are
     bounded); 32x32 stream-transpose + SBUF-SBUF DMA reassembly into
     pair-major score rows [pair, 17664]
  3. chunk-max (L=8) -> M [pair, 2208]; chunk ids are packed into the low
     12 mantissa bits of M (bit-complemented so exact ties keep ascending
     chunk order), so 25 rounds of max8/match_replace extract both the 200
     largest chunk maxes AND their ids with no find_index8 pass. The
     packing quantization (2^-11 relative) only perturbs which chunks make
     the top-200 cut at the boundary — scores themselves stay exact.
  4. indirect-DMA gather of those chunks -> pool [pair, 1600]; 25 exact
     max8/max_index/match_replace rounds give the sorted top-200 + their
     pool slots
  5. pool slot -> chunk id via a u16 one-hot select; box rows arrive by
     per-rank indirect gathers
  6. upper-triangle IoU mask (iou > 0.45 as 1.45*inter - 0.45*(ai+aj) > 0),
     emitted two sort-rounds behind the gathers so the DVE never waits
  7. greedy NMS via a running suppression vector: keep[i] = (supv[i]==0);
     supv[k>i] max= keep[i]*Sm[i, k>i]
  8. sorted rows + keep flags DMA out; the host compacts survivors
     (replaces 400 per-rank scatter DMAs that dominated the old tail)

The 168 pairs exceed the 128 SBUF partitions, so phase 2 runs as two
partition tiles (126 + 42 pairs) whose instruction streams interleave
op-by-op to hide dependency latency.
"""

import numpy as np

B = 64
N = 8732
C = 21
NPAD = 8832            # priors padded to 69*128
SLOTS = NPAD // 128    # 69
W2 = 2 * NPAD          # 17664 score columns per pair
IMG = 8                # images per core
PAIRS = IMG * C        # 168 pairs per core
CHUNK = 8
NCHUNK = W2 // CHUNK   # 2208
ROUNDS = 25            # 25*8 = 200
K = 200
SKEW = 2               # IoU runs this many sort-rounds behind the gathers
TILES = ((0, 126), (126, 42))   # (pair offset, pair count) per partition tile


def build_nc(level=99):
    import concourse.bacc as bacc
    import concourse.bass as bass
    import concourse.mybir as mybir
    from concourse.bass import IndirectOffsetOnAxis
    from concourse.tile import TileContext

    f32 = mybir.dt.float32
    u32 = mybir.dt.uint32
    u16 = mybir.dt.uint16
    u8 = mybir.dt.uint8
    Alu = mybir.AluOpType
    Act = mybir.ActivationFunctionType
    Ax = mybir.AxisListType

    nc = bacc.Bacc()

    def TT(out, in0, in1, op):
        # TensorTensor's ISA struct can't encode multiple sync waits; emit
        # every tensor-tensor op as (in0 bypass 0.0) op in1 instead.
        nc.vector.scalar_tensor_tensor(
            out=out, in0=in0, scalar=0.0, in1=in1, op0=Alu.bypass, op1=op,
        )

    loc1 = nc.declare_dram_parameter("loc1", [IMG, NPAD, 4], f32, isOutput=False)
    loc2 = nc.declare_dram_parameter("loc2", [IMG, NPAD, 4], f32, isOutput=False)
    conf1 = nc.declare_dram_parameter("conf1", [IMG, NPAD, C], f32, isOutput=False)
    conf2 = nc.declare_dram_parameter("conf2", [IMG, NPAD, C], f32, isOutput=False)
    dbox = nc.declare_dram_parameter("dbox", [NPAD, 4], f32, isOutput=False)
    # aux[pair] = (img*W2 base row into box table, unused)
    aux = nc.declare_dram_parameter("aux", [PAIRS, 2], u32, isOutput=False)
    outs = nc.declare_dram_parameter("outs", [PAIRS, K], f32, isOutput=True)
    outb = nc.declare_dram_parameter("outb", [PAIRS, K, 4], f32, isOutput=True)
    outk = nc.declare_dram_parameter("outk", [PAIRS, K], u8, isOutput=True)

    scoresD = nc.dram_tensor("scoresD", [PAIRS * NCHUNK, CHUNK], f32)
    # prior-ordered box table: row = (img*2+v)*NPAD + prior
    boxesD = nc.dram_tensor("boxesD", [IMG * W2, 4], f32)

    with TileContext(nc) as tc:
        with tc.tile_pool(name="Mpool", bufs=1) as mp:
            M_tiles = [
                mp.tile([cnt, NCHUNK], f32, tag=f"M{ti}", name=f"M{ti}")
                for ti, (off, cnt) in enumerate(TILES)
            ]
            # ---------------- phase 1: scores + boxes -----------------------
            with (
                tc.tile_pool(name="persist", bufs=1) as pp,
                tc.tile_pool(name="work", bufs=1) as wp,
                tc.tile_pool(name="workdb", bufs=2) as wdb,
            ):
                SA = pp.tile([126, W2], f32, tag="SA")
                SB = pp.tile([42, W2], f32, tag="SB")
                S_tiles = (SA, SB)

                pr_t = wp.tile([128, SLOTS, 32], f32, tag="pr")
                # pad columns feed transpose lanes that are never read, but
                # must not be uninitialized SBUF
                nc.gpsimd.memset(pr_t[:, :, C:], 0.0)

                dbox_t = pp.tile([128, SLOTS, 4], f32, tag="dbox")
                nc.sync.dma_start(
                    out=dbox_t[:, :, :],
                    in_=dbox.rearrange("(p s) c -> p s c", s=SLOTS),
                )

                for img in range(IMG):
                    for v, (locp, confp) in enumerate(
                        ((loc1, conf1), (loc2, conf2))
                    ):
                        # ---- decode ----
                        loc_t = wdb.tile([128, SLOTS, 4], f32, tag="loc")
                        nc.sync.dma_start(
                            out=loc_t[:, :, :],
                            in_=locp[img].rearrange("(p s) c -> p s c", s=SLOTS),
                        )
                        box_t = wdb.tile([128, SLOTS, 4], f32, tag="box")
                        wh_t = wp.tile([128, SLOTS, 2], f32, tag="wh")
                        t1_t = wp.tile([128, SLOTS, 2], f32, tag="dtmp")
                        # wh = dbox_wh * exp(0.2*loc_wh)
                        nc.scalar.activation(
                            wh_t[:, :, :], loc_t[:, :, 2:4], Act.Exp, scale=0.2
                        )
                        TT(
                            out=wh_t[:, :, :], in0=wh_t[:, :, :],
                            in1=dbox_t[:, :, 2:4], op=Alu.mult,
                        )
                        # cxy = dbox_xy + (loc_xy*0.1)*dbox_xy
                        nc.vector.tensor_scalar_mul(
                            t1_t[:, :, :], loc_t[:, :, :2], 0.1
                        )
                        TT(
                            out=t1_t[:, :, :], in0=t1_t[:, :, :],
                            in1=dbox_t[:, :, :2], op=Alu.mult,
                        )
                        TT(
                            out=t1_t[:, :, :], in0=t1_t[:, :, :],
                            in1=dbox_t[:, :, :2], op=Alu.add,
                        )
                        # mn = cxy - 0.5*wh ; mx = mn + wh
                        nc.vector.tensor_scalar_mul(
                            box_t[:, :, 2:4], wh_t[:, :, :], 0.5
                        )
                        TT(
                            out=box_t[:, :, 0:2], in0=t1_t[:, :, :],
                            in1=box_t[:, :, 2:4], op=Alu.subtract,
                        )
                        TT(
                            out=box_t[:, :, 2:4], in0=box_t[:, :, 0:2],
                            in1=wh_t[:, :, :], op=Alu.add,
                        )
                        if v == 1:
                            # flip: x1' = 1-x2, x2' = 1-x1
                            fx_t = wp.tile([128, SLOTS, 2], f32, tag="fx")
                            nc.vector.tensor_scalar(
                                fx_t[:, :, 0:1], box_t[:, :, 2:3], -1.0, 1.0,
                                op0=Alu.mult, op1=Alu.add,
                            )
                            nc.vector.tensor_scalar(
                                fx_t[:, :, 1:2], box_t[:, :, 0:1], -1.0, 1.0,
                                op0=Alu.mult, op1=Alu.add,
                            )
                            nc.vector.tensor_copy(
                                box_t[:, :, 0:1], fx_t[:, :, 0:1]
                            )
                            nc.vector.tensor_copy(
                                box_t[:, :, 2:3], fx_t[:, :, 1:2]
                            )
                        # boxesD rows (img*2+v)*NPAD + prior: contiguous per
                        # partition (prior = p*SLOTS + s), one clean DMA
                        nc.scalar.dma_start(
                            out=boxesD[(img * 2 + v) * NPAD:
                                       (img * 2 + v + 1) * NPAD, :].rearrange(
                                "(p s) c -> p s c", s=SLOTS
                            ),
                            in_=box_t[:, :, :],
                        )

                        # ---- softmax (prior-major) ----
                        cf_t = wdb.tile([128, SLOTS, C], f32, tag="cf")
                        nc.scalar.dma_start(
                            out=cf_t[:, :, :],
                            in_=confp[img].rearrange("(p s) c -> p s c", s=SLOTS),
                        )
                        mx_t = wp.tile([128, SLOTS], f32, tag="mx")
                        nc.vector.tensor_reduce(
                            out=mx_t[:, :], in_=cf_t[:, :, :], axis=Ax.X,
                            op=Alu.max,
                        )
                        TT(
                            out=cf_t[:, :, :], in0=cf_t[:, :, :],
                            in1=mx_t[:, :, None].to_broadcast([128, SLOTS, C]),
                            op=Alu.subtract,
                        )
                        nc.scalar.activation(
                            pr_t[:, :, :C], cf_t[:, :, :], Act.Exp
                        )
                        sm_t = wp.tile([128, SLOTS], f32, tag="sm")
                        nc.vector.tensor_reduce(
                            out=sm_t[:, :], in_=pr_t[:, :, :C], axis=Ax.X,
                            op=Alu.add,
                        )
                        nc.vector.reciprocal(sm_t[:, :], sm_t[:, :])
                        TT(
                            out=pr_t[:, :, :C], in0=pr_t[:, :, :C],
                            in1=sm_t[:, :, None].to_broadcast([128, SLOTS, C]),
                            op=Alu.mult,
                        )
                        # ---- 32x32 block transpose ----
                        tr_t = wdb.tile([128, SLOTS, 32], f32, tag="tr")
                        nc.vector.transpose(
                            out=tr_t[:, :, :].rearrange("p s c -> p (s c)"),
                            in_=pr_t[:, :, :].rearrange("p s c -> p (s c)"),
                        )
                        # ---- SBUF->SBUF DMA into pair-major rows ----
                        if img < 6:
                            dst, row0 = SA, img * C
                        else:
                            dst, row0 = SB, (img - 6) * C
                        for pg in range(4):
                            nc.sync.dma_start(
                                out=dst[row0:row0 + C,
                                        v * NPAD + pg * (SLOTS * 32):
                                        v * NPAD + (pg + 1) * (SLOTS * 32)],
                                in_=tr_t[pg * 32: pg * 32 + C, :, :].rearrange(
                                    "c s l -> c (s l)"
                                ),
                            )

                # big copy of pair-major scores to DRAM + chunk max
                for (off, cnt), st, M_t in zip(TILES, S_tiles, M_tiles):
                    nc.scalar.dma_start(
                        out=scoresD[off * NCHUNK:(off + cnt) * NCHUNK, :],
                        in_=st[:, :].rearrange("p (n k) -> p n k", k=CHUNK),
                    )
                    nc.vector.tensor_reduce(
                        out=M_t[:, :],
                        in_=st[:, :].rearrange("p (n k) -> p n k", k=CHUNK),
                        axis=Ax.X, op=Alu.max,
                    )
            # persist pool (SA/SB) freed here

            # -------- phase 2: selection + NMS, tiles interleaved -----------
            if level < 2:
                nc.compile()
                return nc
            with tc.tile_pool(name="sel", bufs=1) as sp:
                D = []   # per-tile working tiles
                for ti, (off, cnt) in enumerate(TILES):
                    d = {}
                    d["cids"] = sp.tile([cnt, K], u32, tag=f"ci{ti}", name=f"ci{ti}")
                    d["pb"] = sp.tile([cnt, K], u16, tag=f"pb{ti}", name=f"pb{ti}")
                    d["t0"] = sp.tile([cnt, K], u32, tag=f"t0{ti}", name=f"t0{ti}")
                    d["t1"] = sp.tile([cnt, K], u32, tag=f"t1{ti}", name=f"t1{ti}")
                    d["t2"] = sp.tile([cnt, K], u32, tag=f"t2{ti}", name=f"t2{ti}")
                    d["t3"] = sp.tile([cnt, K], u32, tag=f"t3{ti}", name=f"t3{ti}")
                    d["c32"] = sp.tile([cnt, 8], u32, tag=f"c3{ti}", name=f"c3{ti}")
                    d["v8"] = sp.tile([cnt, 8], f32, tag=f"v8{ti}", name=f"v8{ti}")
                    # f32 per-partition bases (values < 2^24, exact): AP
                    # scalars for arithmetic tensor_scalar ops must be f32
                    d["base"] = sp.tile([cnt, 1], f32, tag=f"ba{ti}", name=f"ba{ti}")
                    nc.gpsimd.iota(
                        d["base"][:, :], pattern=[[0, 1]],
                        base=off * NCHUNK, channel_multiplier=NCHUNK,
                        allow_small_or_imprecise_dtypes=True,
                    )
                    # descending 12-bit id pattern for the mantissa packing
                    d["revi"] = sp.tile([cnt, NCHUNK], u32, tag=f"rv{ti}", name=f"rv{ti}")
                    nc.gpsimd.iota(
                        d["revi"][:, :], pattern=[[-1, NCHUNK]],
                        base=4095, channel_multiplier=0,
                    )
                    d["iot"] = sp.tile([cnt, K], u16, tag=f"io{ti}", name=f"io{ti}")
                    nc.gpsimd.iota(
                        d["iot"][:, :], pattern=[[1, K]], base=0,
                        channel_multiplier=0,
                    )
                    d["gidx"] = sp.tile([cnt, K], u32, tag=f"gi{ti}", name=f"gi{ti}")
                    d["pool"] = sp.tile([cnt, K, CHUNK], f32, tag=f"po{ti}", name=f"po{ti}")
                    d["aux"] = sp.tile([cnt, 2], u32, tag=f"ax{ti}", name=f"ax{ti}")
                    nc.sync.dma_start(
                        out=d["aux"][:, :], in_=aux[off:off + cnt, :]
                    )
                    d["auxf"] = sp.tile([cnt, 1], f32, tag=f"af{ti}", name=f"af{ti}")
                    nc.vector.tensor_copy(d["auxf"][:, :], d["aux"][:, 0:1])
                    D.append(d)

                # ---- pack chunk ids into the low 12 mantissa bits of M ----
                for ti, (off, cnt) in enumerate(TILES):
                    M_t, d = M_tiles[ti], D[ti]
                    Mu = M_t[:, :].bitcast(u32)
                    nc.vector.tensor_scalar(
                        Mu, Mu, 12, 12,
                        op0=Alu.logical_shift_right, op1=Alu.logical_shift_left,
                    )
                    nc.vector.tensor_tensor(
                        out=Mu, in0=Mu, in1=d["revi"][:, :], op=Alu.bitwise_or
                    )

                # ---- top-200 chunk extraction, pool gather fused in ----
                # tile1 runs completely first: its 200 pool gathers clear the
                # Q7 early so its sort/IoU/scan overlap tile0's gather backlog
                for r in range(ROUNDS):
                    for ti in (1, 0):
                        off, cnt = TILES[ti]
                        M_t, d = M_tiles[ti], D[ti]
                        nc.vector.max(out=d["v8"][:, :], in_=M_t[:, :])
                        nc.vector.match_replace(
                            out=M_t[:, :], in_to_replace=d["v8"][:, :],
                            in_values=M_t[:, :], imm_value=-1.0,
                        )
                        # cid = (v8 & 0xFFF) ^ 0xFFF  (undo the complement)
                        nc.vector.tensor_scalar(
                            d["cids"][:, 8 * r:8 * r + 8],
                            d["v8"][:, :].bitcast(u32), 0xFFF, 0xFFF,
                            op0=Alu.bitwise_and, op1=Alu.bitwise_xor,
                        )
                        nc.vector.tensor_scalar(
                            d["gidx"][:, 8 * r:8 * r + 8],
                            d["cids"][:, 8 * r:8 * r + 8],
                            d["base"][:, :], None, op0=Alu.add,
                        )
                        if ti == 1:
                            # tile0's pool gathers are deferred into tile1's
                            # sort loop so tile1's box gathers aren't stuck
                            # behind them in the in-order Q7 stream
                            for j in range(8):
                                sg = 8 * r + j
                                nc.gpsimd.indirect_dma_start(
                                    out=d["pool"][:, sg, :], out_offset=None,
                                    in_=scoresD[:, :],
                                    in_offset=IndirectOffsetOnAxis(
                                        ap=d["gidx"][:, sg:sg + 1], axis=0),
                                )
                if level < 3:
                    nc.compile()
                    return nc

                # ---- chunk id -> prior-base decomposition (pb) ----
                # cid = v*1104 + pg*276 + s*4 + l38; box row =
                # imgv*NPAD + (pg*32 + l38*8 + lane)*69 + s, so
                # pb = v*NPAD + pg*2208 + l38*552 + s  (fits u16)
                for ti, (off, cnt) in enumerate(TILES):
                    d = D[ti]

                    def ts(out_, in_, s1, s2, op0, op1=None):
                        kw = {} if op1 is None else {"op1": op1}
                        nc.vector.tensor_scalar(out_, in_, s1, s2, op0=op0, **kw)

                    def stt(out_, in0, s, in1, op0, op1):
                        nc.vector.scalar_tensor_tensor(
                            out=out_, in0=in0, scalar=s, in1=in1,
                            op0=op0, op1=op1,
                        )

                    cids = d["cids"][:, :]
                    t0, t1, t2, t3 = (d[k][:, :] for k in ("t0", "t1", "t2", "t3"))
                    ts(t0, cids, 1104, None, Alu.is_ge)                  # v
                    stt(t1, t0, -1104.0, cids, Alu.mult, Alu.add)        # c2
                    ts(t2, t1, 276, None, Alu.is_ge)                     # p1
                    ts(t3, t1, 552, None, Alu.is_ge)                     # p2
                    TT(out=t2, in0=t2, in1=t3, op=Alu.add)
                    ts(t3, t1, 828, None, Alu.is_ge)                     # p3
                    TT(out=t2, in0=t2, in1=t3, op=Alu.add)               # pg
                    stt(t1, t2, -276.0, t1, Alu.mult, Alu.add)           # c3
                    ts(t3, t1, 2, None, Alu.logical_shift_right)         # s
                    ts(t1, t1, 3, None, Alu.bitwise_and)                 # l38
                    stt(t1, t1, 552.0, t3, Alu.mult, Alu.add)            # +s
                    stt(t1, t2, 2208.0, t1, Alu.mult, Alu.add)           # +pg
                    stt(t1, t0, float(NPAD), t1, Alu.mult, Alu.add)      # +v
                    nc.vector.tensor_copy(d["pb"][:, :], t1)

                # ---- fused: pool sort -> one-hot sigma -> box gather -> IoU
                for ti, (off, cnt) in enumerate(TILES):
                    d = D[ti]
                    d["sorted"] = sp.tile([cnt, K], f32, tag=f"so{ti}", name=f"so{ti}")
                    d["ps"] = sp.tile([cnt, K], u32, tag=f"ps{ti}", name=f"ps{ti}")
                    d["sh"] = sp.tile([cnt, K], u16, tag=f"sh{ti}", name=f"sh{ti}")
                    d["oh"] = sp.tile([cnt, 8, K], u16, tag=f"oh{ti}", name=f"oh{ti}")
                    d["ohm"] = sp.tile([cnt, 8, K], u16, tag=f"om{ti}", name=f"om{ti}")
                    d["csel"] = sp.tile([cnt, K], u16, tag=f"cs{ti}", name=f"cs{ti}")
                    d["lane"] = sp.tile([cnt, K], u32, tag=f"la{ti}", name=f"la{ti}")
                    d["sig"] = sp.tile([cnt, K], u32, tag=f"sg{ti}", name=f"sg{ti}")
                    d["bx"] = sp.tile([cnt, K, 4], f32, tag=f"bx{ti}", name=f"bx{ti}")
                    d["ar"] = sp.tile([cnt, K], f32, tag=f"ar{ti}", name=f"ar{ti}")
                    d["w0"] = sp.tile([cnt, K], f32, tag=f"w0{ti}", name=f"w0{ti}")
                    d["Sm"] = sp.tile([cnt, K, K], u8, tag=f"Sm{ti}", name=f"Sm{ti}")
                    d["xa"] = sp.tile([cnt, K, 8], f32, tag=f"xa{ti}", name=f"xa{ti}")
                    d["xb"] = sp.tile([cnt, K, 8], f32, tag=f"xb{ti}", name=f"xb{ti}")
                    d["xc"] = sp.tile([cnt, K, 8], f32, tag=f"xc{ti}", name=f"xc{ti}")

                def emit_ar_iou(ti, rq):
                    # areas + IoU column-block for sort round rq
                    q0, q1 = 8 * rq, 8 * rq + 8
                    if True:
                        off, cnt = TILES[ti]
                        d = D[ti]
                        bxg = d["bx"][:, q0:q1, :]
                        TT(
                            out=d["w0"][:, q0:q1], in0=bxg[:, :, 2],
                            in1=bxg[:, :, 0], op=Alu.subtract,
                        )
                        TT(
                            out=d["ar"][:, q0:q1], in0=bxg[:, :, 3],
                            in1=bxg[:, :, 1], op=Alu.subtract,
                        )
                        TT(
                            out=d["ar"][:, q0:q1], in0=d["ar"][:, q0:q1],
                            in1=d["w0"][:, q0:q1], op=Alu.mult,
                        )
                        # IoU column-block: S[a, b] for a < r1, b in [r0, r1)
                        A = q1
                        sh3 = [cnt, A, 8]
                        bx = d["bx"]
                        xa = d["xa"][:, :A, :]
                        xb = d["xb"][:, :A, :]
                        xc = d["xc"][:, :A, :]
                        TT(
                            out=xc,
                            in0=bx[:, :A, 1:2].to_broadcast(sh3),
                            in1=bx[:, None, q0:q1, 1].to_broadcast(sh3),
                            op=Alu.max,
                        )
                        TT(
                            out=xb,
                            in0=bx[:, :A, 3:4].to_broadcast(sh3),
                            in1=bx[:, None, q0:q1, 3].to_broadcast(sh3),
                            op=Alu.min,
                        )
                        TT(out=xb, in0=xb, in1=xc, op=Alu.subtract)
                        nc.scalar.activation(xb, xb, Act.Relu)
                        TT(
                            out=xa,
                            in0=bx[:, :A, 0:1].to_broadcast(sh3),
                            in1=bx[:, None, q0:q1, 0].to_broadcast(sh3),
                            op=Alu.max,
                        )
                        TT(
                            out=xc,
                            in0=bx[:, :A, 2:3].to_broadcast(sh3),
                            in1=bx[:, None, q0:q1, 2].to_broadcast(sh3),
                            op=Alu.min,
                        )
                        TT(out=xa, in0=xc, in1=xa, op=Alu.subtract)
                        nc.scalar.activation(xa, xa, Act.Relu)
                        TT(out=xa, in0=xa, in1=xb, op=Alu.mult)
                        TT(
                            out=xb,
                            in0=d["ar"][:, :A, None].to_broadcast(sh3),
                            in1=d["ar"][:, None, q0:q1].to_broadcast(sh3),
                            op=Alu.add,
                        )
                        nc.vector.scalar_tensor_tensor(
                            out=xa, in0=xb, scalar=-0.45 / 1.45, in1=xa,
                            op0=Alu.mult, op1=Alu.add,
                        )
                        nc.vector.tensor_scalar(
                            d["Sm"][:, :A, q0:q1], xa, 0.0, None,
                            op0=Alu.is_gt,
                        )

                # NMS scan tiles (allocated up front; tile1's scan
                # interleaves into tile0's sort rounds)
                for ti, (off, cnt) in enumerate(TILES):
                    d = D[ti]
                    d["keep"] = sp.tile([cnt, K], u8, tag=f"ke{ti}", name=f"ke{ti}")
                    d["supv"] = sp.tile([cnt, K], u8, tag=f"sv{ti}", name=f"sv{ti}")
                    nc.vector.memset(d["supv"][:, :], 0)

                def emit_scan_steps(ti, steps):
                    d = D[ti]
                    for i in steps:
                        nc.vector.scalar_tensor_tensor(
                            out=d["supv"][:, i + 1:],
                            in0=d["Sm"][:, i, i + 1:],
                            scalar=d["supv"][:, i:i + 1],
                            in1=d["supv"][:, i + 1:],
                            op0=Alu.is_gt, op1=Alu.max,
                        )

                def emit_sort_round(ti, r):
                    r0, r1 = 8 * r, 8 * r + 8
                    # sort round r only needs pool slots < 64r+64: rank
                    # k's chunk has chunk-max rank <= k, i.e. slot < 8k+8.
                    # +24 chunks of margin absorb rank perturbation from the
                    # mantissa packing (2^-11) and exact-tie groups.
                    npre = min(K, 8 * r + 8 + 24)
                    off, cnt = TILES[ti]
                    d = D[ti]
                    poolf = d["pool"][:, :npre, :].rearrange(
                        "p n k -> p (n k)")
                    nc.vector.max(
                        out=d["sorted"][:, r0:r1], in_=poolf
                    )
                    nc.vector.max_index(
                        out=d["ps"][:, r0:r1],
                        in_max=d["sorted"][:, r0:r1], in_values=poolf,
                    )
                    nc.vector.match_replace(
                        out=poolf,
                        in_to_replace=d["sorted"][:, r0:r1],
                        in_values=poolf, imm_value=-1.0,
                    )
                    # slot -> chunk id (u16 one-hot over the 200 slots);
                    # bitvec shift can't cast u32->u16, so shift then copy
                    nc.vector.tensor_scalar(
                        d["c32"][:, :], d["ps"][:, r0:r1], 3, None,
                        op0=Alu.logical_shift_right,
                    )
                    nc.vector.tensor_copy(
                        d["sh"][:, r0:r1], d["c32"][:, :]
                    )
                    sh3 = [cnt, 8, K]
                    TT(
                        out=d["oh"][:, :, :],
                        in0=d["sh"][:, r0:r1, None].to_broadcast(sh3),
                        in1=d["iot"][:, None, :].to_broadcast(sh3),
                        op=Alu.is_equal,
                    )
                    TT(
                        out=d["ohm"][:, :, :], in0=d["oh"][:, :, :],
                        in1=d["pb"][:, None, :].to_broadcast(sh3),
                        op=Alu.mult,
                    )
                    nc.vector.tensor_reduce(
                        out=d["csel"][:, r0:r1], in_=d["ohm"][:, :, :],
                        axis=Ax.X, op=Alu.max,
                    )
                    # box row = imgv*NPAD + pb + lane*69
                    nc.vector.tensor_scalar(
                        d["lane"][:, r0:r1], d["ps"][:, r0:r1], 7, None,
                        op0=Alu.bitwise_and,
                    )
                    nc.vector.scalar_tensor_tensor(
                        out=d["sig"][:, r0:r1], in0=d["lane"][:, r0:r1],
                        scalar=float(SLOTS), in1=d["csel"][:, r0:r1],
                        op0=Alu.mult, op1=Alu.add,
                    )
                    nc.vector.tensor_scalar(
                        d["sig"][:, r0:r1], d["sig"][:, r0:r1],
                        d["auxf"][:, :], None, op0=Alu.add,
                    )
                    # box gathers queue as soon as this tile's sig lands
                    for sg in range(r0, r1):
                        nc.gpsimd.indirect_dma_start(
                            out=d["bx"][:, sg, :], out_offset=None,
                            in_=boxesD[:, :],
                            in_offset=IndirectOffsetOnAxis(
                                ap=d["sig"][:, sg:sg + 1], axis=0),
                        )

                def finish_tile(ti):
                    off, cnt = TILES[ti]
                    d = D[ti]
                    nc.sync.dma_start(
                        out=outs[off:off + cnt, :], in_=d["sorted"][:, :]
                    )
                    nc.scalar.dma_start(
                        out=outb[off:off + cnt, :, :], in_=d["bx"][:, :, :]
                    )

                def finish_scan(ti):
                    off, cnt = TILES[ti]
                    d = D[ti]
                    nc.vector.tensor_scalar(
                        d["keep"][:, :], d["supv"][:, :], 0, None,
                        op0=Alu.is_equal,
                    )
                    nc.sync.dma_start(
                        out=outk[off:off + cnt, :], in_=d["keep"][:, :]
                    )

                # merged 3-stream schedule: per super-round, tile0's pool
                # gathers lead (they never stall on DVE), then tile1's sort+
                # box gathers+IoU, then tile0's streams shifted 5 rounds back
                # so its pool data has arrived by the time its sort issues.
                d0 = D[0]
                T0LAG = 5
                for r in range(ROUNDS + T0LAG + SKEW):
                    if r < ROUNDS:
                        for sg in range(8 * r, 8 * r + 8):
                            nc.gpsimd.indirect_dma_start(
                                out=d0["pool"][:, sg, :], out_offset=None,
                                in_=scoresD[:, :],
                                in_offset=IndirectOffsetOnAxis(
                                    ap=d0["gidx"][:, sg:sg + 1], axis=0),
                            )
                        emit_sort_round(1, r)
                    if SKEW <= r < ROUNDS + SKEW:
                        emit_ar_iou(1, r - SKEW)
                    q = r - T0LAG
                    if 0 <= q < ROUNDS:
                        emit_sort_round(0, q)
                    q2 = r - T0LAG - SKEW
                    if 0 <= q2 < ROUNDS:
                        emit_ar_iou(0, q2)
                finish_tile(1)
                finish_tile(0)
                if level < 6:
                    nc.compile()
                    return nc

                # NMS scans, two chains interleaved for latency hiding
                for i in range(K - 1):
                    emit_scan_steps(1, [i])
                    emit_scan_steps(0, [i])
                finish_scan(1)
                finish_scan(0)
    nc.compile()
    return nc


def _prep_core_inputs(loc_b, conf_b, loc2_b, conf2_b, dbox):
    """Pad per-core inputs to NPAD priors; build aux table."""
    pad = NPAD - N
    locp = np.pad(loc_b, ((0, 0), (0, pad), (0, 0)))
    loc2p = np.pad(loc2_b, ((0, 0), (0, pad), (0, 0)))
    cpad = np.zeros((conf_b.shape[0], pad, C), np.float32)
    cpad[:, :, 0] = 40.0
    cpad[:, :, 1:] = -40.0
    confp = np.concatenate([conf_b, cpad], axis=1)
    conf2p = np.concatenate([conf2_b, cpad], axis=1)
    dpad = np.zeros((pad, 4), np.float32)
    dpad[:, 2:] = 1e-3
    dboxp = np.concatenate([dbox, dpad], axis=0)
    aux = np.zeros((PAIRS, 2), np.uint32)
    for p in range(PAIRS):
        aux[p, 0] = (p // C) * W2
    return {
        "loc1": np.ascontiguousarray(locp, np.float32),
        "loc2": np.ascontiguousarray(loc2p, np.float32),
        "conf1": np.ascontiguousarray(confp, np.float32),
        "conf2": np.ascontiguousarray(conf2p, np.float32),
        "dbox": np.ascontiguousarray(dboxp, np.float32),
        "aux": aux,
    }


def _compact_core(outs, outb, outk):
    """Host-side NMS survivor compaction for one core's outputs."""
    rows = np.concatenate([outs[:, :, None], outb], axis=2)  # [PAIRS, K, 5]
    keep = outk.astype(bool)
    keep[0::C, :] = False   # background class never processed
    pos = np.cumsum(keep, axis=1) - 1
    out = np.zeros((PAIRS, K, 5), np.float32)
    pi, ri = np.nonzero(keep)
    out[pi, pos[keep], :] = rows[pi, ri, :]
    return out


def kernel(loc_data, conf_data, loc_data2, conf_data2, dbox_list):
    from concourse.bass_utils import run_bass_kernel_spmd

    loc_data = np.asarray(loc_data, np.float32)
    conf_data = np.asarray(conf_data, np.float32)
    loc_data2 = np.asarray(loc_data2, np.float32)
    conf_data2 = np.asarray(conf_data2, np.float32)
    dbox_list = np.asarray(dbox_list, np.float32)

    nc = build_nc()
    in_maps = []
    for k in range(8):
        sl = slice(k * IMG, (k + 1) * IMG)
        in_maps.append(
            _prep_core_inputs(
                loc_data[sl], conf_data[sl], loc_data2[sl], conf_data2[sl],
                dbox_list,
            )
        )
    res = run_bass_kernel_spmd(nc, in_maps, list(range(8))).results
    outs = []
    for k in range(8):
        o = _compact_core(
            np.asarray(res[k]["outs"]), np.asarray(res[k]["outb"]),
            np.asarray(res[k]["outk"]),
        )
        outs.append(o.reshape(IMG, C, K, 5))
    return np.concatenate(outs, axis=0)
